# revision 24
# baseline (speedup 1.0000x reference)
"""Bass/Trainium2 kernel for a 4-layer GraphSAGE GNN (mean aggregation).

Problem (hardcoded): N=100000 nodes, E=1200000 edges, x:[N,3] f32,
edge_index:[2,E] int64, hidden=64, out=2, log_softmax output.

  h1 = relu(mean_nbr(x) @ Wl1 + x @ Wr1 + b1)
  h2 = relu(mean_nbr(h1) @ Wl2 + h1 @ Wr2 + b2)
  h3 = relu(mean_nbr(h2) @ Wl3 + h2 @ Wr3 + b3)
  out = log_softmax(mean_nbr(h3) @ Wl4 + h3 @ Wr4 + b4)

Strategy v3 (8 NeuronCores, node-partitioned, pair-gather + DVE agg):
- Core k owns nodes [k*12544, (k+1)*12544), permuted within the core so
  each 128-node dst tile is degree-homogeneous (sort by (deg, chunk0
  in-count) within frozen half-shards).
- Per layer the node table (node-major [*, 64] f32) is allgathered in
  two halves T_a (all cores' first 6144 permuted rows) and T_b (rest):
  pair rows (two nodes = 512B) are dma_gather'ed at full HBM bandwidth
  (256B rows run at ~110GB/s, 512B at ~380GB/s).
- Gather streams are slot-aligned: edge for dst slot s, round j of tile
  t sits at stream position (S_t + j)*128 + s, so the gathered tile
  slice j holds dst slot s's round-j pair on partition s. Aggregation is
  tmp = pair_half * mask (DVE tensor_tensor, mask has invdeg / parity
  selection / zero padding folded in, persistent in SBUF), then one
  strided tensor_reduce per tile -> node-major mean [128, 64]. PE only
  transposes that to feature-major for the epilogue. No selection-matrix
  (Sw) traffic, no per-edge matmuls.
- Epilogue per 512 nodes: 3 PSUM matmuls (Wl/Wr/bias-rank-1), relu on
  ACT; h kept feature-major in a DRAM ping-pong for the Wr matmul and
  node-major in the half-tables for the next layer's gather. The half
  split lets AllGather(T_a) overlap the second half's epilogue and the
  chunk-1 gathers of the next layer.
- Shared schedule across cores (J = max over cores per (tile, chunk));
  shorter cores pad with token-0 gathers killed by zero masks.
"""

import os
import numpy as np
from contextlib import ExitStack

_SKIP_CC = os.environ.get("K_SKIP_CC", "") != ""
_SKIP_EDGE = os.environ.get("K_SKIP_EDGE", "") != ""
_SKIP_AGG = os.environ.get("K_SKIP_AGG", "") != ""

# ---- problem constants (self-contained; do not read spec/reference) ----
N = 100000
E = 1200000
NCORES = 8
NPC = -(-N // (NCORES * 128)) * 128  # nodes per core = 12544 = 98 * 128
NPAD = NCORES * NPC                  # 100352
NBLK = NPC // 128                    # tiles per core = 98
F = 64
FIN = 3
FOUT = 2
SUB = int(os.environ.get("K_SUB", "1024"))   # gather positions per call
SLICES = SUB // 128                          # slices per call
SCRATCH = 16384 if SUB <= 1024 else 32 * SUB  # SWDGE ring carveout (2 calls/queue)
GROUP = 512                                  # nodes per epilogue group
GT = GROUP // 128                            # tiles per group = 4
# chunk split: per-core permuted rows [0, SPLIT) -> chunk 0 (must be a
# multiple of GROUP so the a/b allgathers align with epilogue groups)
SPLIT = (NBLK // 2 * 128) // GROUP * GROUP   # 6144 rows = 48 tiles
TILES0 = SPLIT // 128

_CACHE = {}


def _wrap_idx(idx: np.ndarray) -> np.ndarray:
    """Stream position i -> idxs[i%16, i//16], replicated for 8 Q7 cores."""
    w = idx.reshape(-1, 16).T.astype(np.int16)
    return np.tile(w, (8, 1))


def _preprocess(x: np.ndarray, edge_index: np.ndarray):
    src = np.asarray(edge_index[0], dtype=np.int64)
    dst = np.asarray(edge_index[1], dtype=np.int64)
    n_real = x.shape[0]

    deg = np.bincount(dst, minlength=NPAD).astype(np.int64)
    invdeg = (1.0 / np.maximum(deg, 1.0)).astype(np.float32)

    split1 = NPC - SPLIT  # rows per core in chunk 1

    # ---- per-core permutation: phase 1 degree sort, freeze chunk halves,
    # phase 2 sort by (deg, chunk0-in-count) within each half ----
    perm = np.empty((NCORES, NPC), np.int64)  # position -> original id
    for k in range(NCORES):
        ids = np.arange(k * NPC, (k + 1) * NPC)
        order1 = np.argsort(-deg[ids], kind="stable")
        perm[k] = ids[order1]
    ischunk0 = np.zeros(NPAD, bool)
    for k in range(NCORES):
        ischunk0[perm[k][:SPLIT]] = True
    c0cnt = np.bincount(dst[ischunk0[src]], minlength=NPAD).astype(np.int64)
    for k in range(NCORES):
        for lo, hi in ((0, SPLIT), (SPLIT, NPC)):
            half = perm[k][lo:hi]
            key = np.lexsort((c0cnt[half], -deg[half]))
            perm[k][lo:hi] = half[key]

    inv_perm = np.empty(NPAD, np.int64)  # original id -> within-core position
    owner_of = np.empty(NPAD, np.int64)
    for k in range(NCORES):
        inv_perm[perm[k]] = np.arange(NPC)
        owner_of[perm[k]] = k

    # ---- edge -> (core, tile, slot, chunk, token, parity) ----
    dst_owner = dst // NPC
    dpos = inv_perm[dst]
    spos = inv_perm[src]
    sowner = owner_of[src]
    schunk = (spos >= SPLIT).astype(np.int64)
    tok = np.where(
        schunk == 0,
        (sowner * SPLIT + spos) >> 1,
        (sowner * split1 + (spos - SPLIT)) >> 1,
    )
    par = np.where(schunk == 0, (sowner * SPLIT + spos) & 1,
                   (sowner * split1 + (spos - SPLIT)) & 1)
    tile = dpos >> 7
    slot = dpos & 127

    # per-core per-(tile, chunk, slot) counts -> shared round counts J
    J = np.zeros((NCORES, NBLK, 2), np.int64)
    per_core = []
    for k in range(NCORES):
        m = dst_owner == k
        key = (tile[m] * 2 + schunk[m]) * 128 + slot[m]
        cnt = np.bincount(key, minlength=NBLK * 2 * 128).reshape(NBLK, 2, 128)
        J[k] = cnt.max(axis=2)
        per_core.append((tile[m], slot[m], schunk[m], tok[m], par[m], dst[m], key))
    Jsh = J.max(axis=0)  # [NBLK, 2] shared schedule

    # slice offsets per chunk and stream lengths
    Soff = np.zeros((NBLK, 2), np.int64)
    Soff[1:] = np.cumsum(Jsh[:-1], axis=0)
    nsl = Jsh.sum(axis=0)  # total slices per chunk
    ncalls = int(max(-(-int(nsl[0]) // SLICES), -(-int(nsl[1]) // SLICES)))
    L = ncalls * SUB

    # mask column layout: per tile [Me0 (J0) | Mo0 (J0) | Me1 (J1) | Mo1 (J1)]
    mbase = np.zeros(NBLK, np.int64)
    mbase[1:] = np.cumsum(2 * (Jsh[:-1, 0] + Jsh[:-1, 1]))
    MCOLS = int(mbase[-1] + 2 * (Jsh[-1, 0] + Jsh[-1, 1]))

    gidx_maps, mask_maps = [], []
    for k in range(NCORES):
        t_k, s_k, c_k, tok_k, par_k, dst_k, key = per_core[k]
        order = np.argsort(key, kind="stable")
        t_k, s_k, c_k, tok_k, par_k, dst_k = (
            t_k[order], s_k[order], c_k[order], tok_k[order], par_k[order],
            dst_k[order],
        )
        key = key[order]
        # round index j = rank within the (tile, chunk, slot) run
        starts = np.zeros(NBLK * 2 * 128, np.int64)
        cnt = np.bincount(key, minlength=NBLK * 2 * 128)
        np.cumsum(cnt[:-1], out=starts[1:])
        j_k = np.arange(len(key)) - starts[key]

        # pad slots spread across the table (token 0 for all pads would
        # funnel ~23% of gather traffic through one HBM channel)
        ntok = (NCORES * SPLIT // 2, NCORES * split1 // 2)
        streams = np.stack(
            [np.arange(L, dtype=np.int64) % max(ntok[c], 1) for c in range(2)]
        )
        pos = (Soff[t_k, c_k] + j_k) * 128 + s_k
        streams[c_k, pos] = tok_k
        gidx_maps.append(_wrap_idx(streams.reshape(-1)))

        msk = np.zeros((128, MCOLS), np.float32)
        col = mbase[t_k] + c_k * 2 * Jsh[t_k, 0] + par_k * Jsh[t_k, c_k] + j_k
        msk[s_k, col] = invdeg[dst_k]
        mask_maps.append(msk)

    # per-core transposed (feature-major) permuted features
    xpad = np.zeros((NPAD, FIN), np.float32)
    xpad[:n_real] = x
    xT = [np.ascontiguousarray(xpad[perm[k]].T) for k in range(NCORES)]

    meta = dict(Jsh=Jsh, Soff=Soff, ncalls=ncalls, L=L, mbase=mbase,
                MCOLS=MCOLS, TM=int(2 * (Jsh[:, 0] + Jsh[:, 1]).max()) + 1)
    return meta, gidx_maps, mask_maps, xT, perm


def _build_module(meta):
    import concourse.bass as bass
    import concourse.bacc as bacc
    import concourse.mybir as mybir
    from concourse import tile
    from concourse import library_config
    from concourse import masks

    f32 = mybir.dt.float32
    i16 = mybir.dt.int16
    AF = mybir.ActivationFunctionType
    ALU = mybir.AluOpType
    AX = mybir.AxisListType

    Jsh, Soff, ncalls, L, mbase, MCOLS, TM = (
        meta["Jsh"], meta["Soff"], meta["ncalls"], meta["L"],
        meta["mbase"], meta["MCOLS"], meta["TM"],
    )
    split1 = NPC - SPLIT
    LG = 2 * L // 16

    nc = bacc.Bacc(
        None,
        target_bir_lowering=False,
        num_swdge_queues=4,
        dynamic_dma_scratch_size=SCRATCH,
    )

    # ---- parameters ----
    xT_p = nc.declare_dram_parameter("xT", [FIN, NPC], f32, isOutput=False)
    gidx_p = nc.declare_dram_parameter("gidx", [128, LG], i16, isOutput=False)
    msk_p = nc.declare_dram_parameter("M", [128, MCOLS], f32, isOutput=False)
    i2_p = nc.declare_dram_parameter("I2", [128, F], f32, isOutput=False)
    wl_p, wr_p, b_p = [None], [None], [None]
    for l in range(1, 5):
        din = FIN if l == 1 else F
        dout = FOUT if l == 4 else F
        wl_p.append(nc.declare_dram_parameter(f"Wl{l}", [din, dout], f32, isOutput=False))
        wr_p.append(nc.declare_dram_parameter(f"Wr{l}", [din, dout], f32, isOutput=False))
        b_p.append(nc.declare_dram_parameter(f"b{l}", [1, dout], f32, isOutput=False))
    out_p = nc.declare_dram_parameter("out_shard", [NPC, FOUT], f32, isOutput=True)

    # ---- internal DRAM: per-layer half tables + shard halves ----
    if SPLIT > 0:
        Ta = [None] + [nc.dram_tensor(f"Ta{l}", [NCORES * SPLIT, F], f32)
                       for l in range(1, 5)]
        sha = [None] + [nc.dram_tensor(f"sha{l}", [SPLIT, F], f32)
                        for l in range(1, 5)]
    else:
        Ta = [None] * 5
        sha = [None] * 5
    Tb = [None] + [nc.dram_tensor(f"Tb{l}", [NCORES * split1, F], f32)
                   for l in range(1, 5)]
    shb = [None] + [nc.dram_tensor(f"shb{l}", [split1, F], f32) for l in range(1, 5)]
    hTd = [nc.dram_tensor(f"hT{i}", [F, NPC], f32) for i in range(2)]

    # epilogue groups: (start_tile, n_tiles); split point falls on a boundary
    egroups = []
    b0 = 0
    while b0 < NBLK:
        nb = min(GT, NBLK - b0)
        egroups.append((b0, nb))
        b0 += nb
    ng_a = TILES0 // GT  # groups belonging to chunk 0

    # a (tile, chunk) run can span several gather calls; its DVE consumers
    # need all of them alive at once -> pool depth must cover the max span
    maxspan = 1
    for t in range(NBLK):
        for c in range(2):
            if Jsh[t, c] == 0:
                continue
            s0 = int(Soff[t, c])
            span = (s0 % SLICES + int(Jsh[t, c]) + SLICES - 1) // SLICES
            maxspan = max(maxspan, span)
    gt_bufs = maxspan + 2

    with tile.TileContext(nc) as tc, ExitStack() as ctx:
        idxp = ctx.enter_context(tc.tile_pool(name="idx", bufs=1))
        constp = ctx.enter_context(tc.tile_pool(name="const", bufs=1))
        gtp = [ctx.enter_context(tc.tile_pool(name=f"gt{c}", bufs=gt_bufs))
               for c in range(2)]
        tmpp = ctx.enter_context(tc.tile_pool(name="tmp", bufs=2))
        grpp = ctx.enter_context(tc.tile_pool(name="grp", bufs=3))
        psT = ctx.enter_context(tc.tile_pool(name="psT", bufs=2, space="PSUM"))
        psB = ctx.enter_context(tc.tile_pool(name="psB", bufs=2, space="PSUM"))
        psC = ctx.enter_context(tc.tile_pool(name="psC", bufs=2, space="PSUM"))

        nc.gpsimd.load_library(library_config.mlp)

        # ---- persistent constants ----
        gi = idxp.tile([128, LG], i16)
        nc.sync.dma_start(gi[:], gidx_p[:])
        msk = idxp.tile([128, MCOLS], f32, tag="msk")
        nc.sync.dma_start(msk[:], msk_p[:])

        ident = constp.tile([128, 128], f32)
        masks.make_identity(nc, ident[:])
        ones = constp.tile([1, GROUP], f32)
        nc.vector.memset(ones[:], 1.0)

        i2_t = constp.tile([128, F], f32, tag="i2")
        nc.sync.dma_start(i2_t[:], i2_p[:])

        wl_t, wls_t, wr_t, b_t = [None], [None], [None], [None]
        for l in range(1, 5):
            din = FIN if l == 1 else F
            dout = FOUT if l == 4 else F
            t1 = constp.tile([din, dout], f32, tag=f"wl{l}")
            t2 = constp.tile([din, dout], f32, tag=f"wr{l}")
            t3 = constp.tile([1, dout], f32, tag=f"b{l}")
            nc.sync.dma_start(t1[:], wl_p[l][:])
            nc.sync.dma_start(t2[:], wr_p[l][:])
            nc.sync.dma_start(t3[:], b_p[l][:])
            wl_t.append(t1)
            wr_t.append(t2)
            b_t.append(t3)
            if l >= 2:
                # stacked [[Wl];[Wl]] so the mean matmul (K=128) folds the
                # PSUM top/bottom-half add from the pair-transpose reduce
                ts = constp.tile([128, dout], f32, tag=f"wls{l}")
                nc.sync.dma_start(ts[:F, :], wl_p[l][:])
                nc.sync.dma_start(ts[F:, :], wl_p[l][:])
                wls_t.append(ts)
            else:
                wls_t.append(None)

        def write_table(l, g0, nb, hnm):
            """DMA node-major h rows [128, nb, F] into the right shard half."""
            rows0 = g0 * 128
            if rows0 < SPLIT:
                dst = sha[l][rows0 : rows0 + nb * 128, :]
            else:
                dst = shb[l][rows0 - SPLIT : rows0 - SPLIT + nb * 128, :]
            nc.sync.dma_start(
                dst.rearrange("(a p) f -> p a f", p=128), hnm[:, :nb, :]
            )

        def allgather(l, half):
            if _SKIP_CC or (half == 0 and SPLIT == 0):
                return
            ins = [sha[l][:]] if half == 0 else [shb[l][:]]
            outs = [Ta[l][:]] if half == 0 else [Tb[l][:]]
            nc.gpsimd.collective_compute(
                "AllGather",
                mybir.AluOpType.bypass,
                replica_groups=[list(range(NCORES))],
                ins=ins,
                outs=outs,
            )

        _REPS = int(os.environ.get("K_REPS", "1"))
        gq = [0]  # global gather emission counter -> DMASW lane consistency
        for _rep in range(_REPS):
            # ---- layer-1 table: T1 = x @ Wl1, node-major, allgather halves ----
            for gidx_g, (g0, nb) in enumerate(egroups):
                xt = grpp.tile([FIN, GROUP], f32, tag="prevT")
                nc.sync.dma_start(
                    xt[:, : nb * 128], xT_p[:, g0 * 128 : (g0 + nb) * 128]
                )
                pnm = psC.tile([128, GT, F], f32, tag="nm")
                for a in range(nb):
                    nc.tensor.matmul(
                        pnm[:, a, :],
                        xt[:, a * 128 : (a + 1) * 128],
                        wl_t[1][:],
                        start=True,
                        stop=True,
                    )
                hnm = grpp.tile([128, GT, F], f32, tag="hnm")
                nc.scalar.activation(hnm[:, :nb, :], pnm[:, :nb, :], AF.Copy)
                write_table(1, g0, nb, hnm)
                if gidx_g == ng_a - 1:
                    allgather(1, 0)
            allgather(1, 1)

            # ---- layers ----
            for l in range(1, 5):
                din = FIN if l == 1 else F
                dout = FOUT if l == 4 else F
                prev_dram = xT_p if l == 1 else hTd[l % 2]
                next_hT = hTd[(l + 1) % 2]

                # pair views of the half tables
                pa = (Ta[l][:].rearrange("(t two) f -> t (two f)", two=2)
                      if SPLIT > 0 else None)
                pb = Tb[l][:].rearrange("(t two) f -> t (two f)", two=2)

                # gather calls, strict queue round-robin c0,c1,c0,c1 ...
                gt_tiles = [[None] * ncalls for _ in range(2)]
                for i in range(ncalls):
                    for c in range(2):
                        if _SKIP_EDGE or (c == 0 and SPLIT == 0):
                            continue
                        gbase = c * L + i * SUB
                        gt = gtp[c].tile([128, SLICES, 2 * F], f32, tag=f"e{c}")
                        nc.gpsimd.dma_gather(
                            gt[:],
                            pa if c == 0 else pb,
                            gi[:, gbase // 16 : (gbase + SUB) // 16],
                            SUB,
                            SUB,
                            2 * F,
                            queue_num=gq[0] % 4,
                        )
                        gq[0] += 1
                        gt_tiles[c][i] = gt

                # aggregation + epilogue per 512-node group
                for gidx_g, (g0, nb) in enumerate(egroups):
                    rows = slice(g0 * 128, (g0 + nb) * 128)
                    mt = grpp.tile([128, GROUP], f32, tag="mt")
                    for a in range(nb):
                        t = g0 + a
                        J0, J1 = int(Jsh[t, 0]), int(Jsh[t, 1])
                        if _SKIP_EDGE or _SKIP_AGG or J0 + J1 == 0:
                            nc.vector.memset(mt[:, a * 128 : (a + 1) * 128], 0.0)
                            continue
                        tmp = tmpp.tile([128, TM, F], f32, tag="tmp")
                        toff = 0
                        for c, Jc in ((0, J0), (1, J1)):
                            if Jc == 0:
                                continue
                            s0 = int(Soff[t, c])
                            for parity in range(2):
                                mc0 = int(
                                    mbase[t] + c * 2 * J0 + parity * Jc
                                )
                                # windows over gather calls
                                j = 0
                                while j < Jc:
                                    ci = (s0 + j) // SLICES
                                    lo = (s0 + j) % SLICES
                                    w = min(SLICES - lo, Jc - j)
                                    gt = gt_tiles[c][ci]
                                    half = slice(parity * F, parity * F + F)
                                    mb = msk[:, mc0 + j : mc0 + j + w].broadcast_to(
                                        [128, w, F]
                                    )
                                    nc.vector.tensor_tensor(
                                        tmp[:, toff : toff + w, :],
                                        gt[:, lo : lo + w, half],
                                        mb,
                                        ALU.mult,
                                    )
                                    toff += w
                                    j += w
                        # feature-major mean via PE: accumulate slice-pair
                        # transposes in PSUM ([0:F] = even slices, [F:] = odd)
                        if toff & 1:
                            nc.vector.memset(tmp[:, toff, :], 0.0)
                            toff += 1
                        pt = psT.tile([128, 128], f32, tag="pt")
                        npairs = toff // 2
                        for j in range(npairs):
                            nc.tensor.matmul(
                                pt[:],
                                tmp[:, 2 * j : 2 * j + 2, :].rearrange(
                                    "p j f -> p (j f)"
                                ),
                                ident[:],
                                start=(j == 0),
                                stop=(j == npairs - 1),
                            )
                        nc.scalar.activation(
                            mt[:, a * 128 : (a + 1) * 128], pt[:], AF.Copy
                        )

                    # prev features (feature-major) for the Wr part
                    pv = grpp.tile([din, GROUP], f32, tag="prevT")
                    nc.sync.dma_start(pv[:, : nb * 128], prev_dram[:, rows])

                    ph = psB.tile([dout, GROUP], f32, tag="h")
                    nc.tensor.matmul(
                        ph[:, : nb * 128],
                        i2_t[:] if l == 1 else wls_t[l][:],
                        mt[:, : nb * 128],
                        start=True,
                        stop=False,
                    )
                    nc.tensor.matmul(
                        ph[:, : nb * 128],
                        wr_t[l][:],
                        pv[:, : nb * 128],
                        start=False,
                        stop=False,
                    )
                    nc.tensor.matmul(
                        ph[:, : nb * 128],
                        b_t[l][:],
                        ones[:, : nb * 128],
                        start=False,
                        stop=True,
                    )

                    if l < 4:
                        hT_sb = grpp.tile([F, GROUP], f32, tag="hT_sb")
                        nc.scalar.activation(
                            hT_sb[:, : nb * 128], ph[:, : nb * 128], AF.Relu
                        )
                        nc.sync.dma_start(next_hT[:, rows], hT_sb[:, : nb * 128])
                        # node-major for the next table
                        pnm = psC.tile([128, GT, F], f32, tag="nm")
                        for a in range(nb):
                            nc.tensor.transpose(
                                pnm[:, a, :],
                                hT_sb[:, a * 128 : (a + 1) * 128],
                                ident[:F, :F],
                            )
                        hnm = grpp.tile([128, GT, F], f32, tag="hnm")
                        nc.vector.tensor_copy(hnm[:, :nb, :], pnm[:, :nb, :])
                        write_table(l + 1, g0, nb, hnm)
                        if gidx_g == ng_a - 1:
                            allgather(l + 1, 0)
                    else:
                        # logits -> node-major -> log_softmax -> out_shard
                        zsb = grpp.tile([FOUT, GROUP], f32, tag="zsb")
                        nc.vector.tensor_copy(zsb[:, : nb * 128], ph[:, : nb * 128])
                        pz = psC.tile([128, GT, FOUT], f32, tag="znm")
                        for a in range(nb):
                            nc.tensor.transpose(
                                pz[:, a, :],
                                zsb[:, a * 128 : (a + 1) * 128],
                                ident[:FOUT, :FOUT],
                            )
                        z = grpp.tile([128, GT, FOUT], f32, tag="z")
                        nc.vector.tensor_copy(z[:, :nb, :], pz[:, :nb, :])
                        z0 = z[:, :nb, 0:1]
                        z1 = z[:, :nb, 1:2]
                        m = grpp.tile([128, GT, 1], f32, tag="m")
                        nc.vector.tensor_tensor(m[:, :nb, :], z0, z1, ALU.max)
                        d = grpp.tile([128, GT, FOUT], f32, tag="d")
                        nc.vector.tensor_tensor(d[:, :nb, 0:1], z0, m[:, :nb, :], ALU.subtract)
                        nc.vector.tensor_tensor(d[:, :nb, 1:2], z1, m[:, :nb, :], ALU.subtract)
                        e = grpp.tile([128, GT, FOUT], f32, tag="e")
                        nc.scalar.activation(e[:, :nb, :], d[:, :nb, :], AF.Exp)
                        s = grpp.tile([128, GT, 1], f32, tag="s")
                        nc.vector.tensor_tensor(
                            s[:, :nb, :], e[:, :nb, 0:1], e[:, :nb, 1:2], ALU.add
                        )
                        ls = grpp.tile([128, GT, 1], f32, tag="ls")
                        nc.scalar.activation(ls[:, :nb, :], s[:, :nb, :], AF.Ln)
                        o = grpp.tile([128, GT, FOUT], f32, tag="o")
                        nc.vector.tensor_tensor(
                            o[:, :nb, 0:1], d[:, :nb, 0:1], ls[:, :nb, :], ALU.subtract
                        )
                        nc.vector.tensor_tensor(
                            o[:, :nb, 1:2], d[:, :nb, 1:2], ls[:, :nb, :], ALU.subtract
                        )
                        nc.sync.dma_start(
                            out_p[rows, :].rearrange("(a p) f -> p a f", p=128),
                            o[:, :nb, :],
                        )

                if l < 4:
                    allgather(l + 1, 1)

    nc.compile()
    return nc


def _prepare(inputs):
    x = np.asarray(inputs["x"], dtype=np.float32)
    edge_index = np.asarray(inputs["edge_index"])
    meta, gidx_maps, mask_maps, xT, perm = _preprocess(x, edge_index)

    key = (
        meta["Jsh"].tobytes(),
        _SKIP_CC,
        _SKIP_EDGE,
        _SKIP_AGG,
        SUB,
        os.environ.get("K_REPS", "1"),
    )
    if key not in _CACHE:
        _CACHE[key] = _build_module(meta)
    nc = _CACHE[key]

    in_maps = []
    for k in range(NCORES):
        m = {
            "xT": xT[k],
            "gidx": gidx_maps[k],
            "M": mask_maps[k],
            "I2": np.tile(np.eye(F, dtype=np.float32), (2, 1)),
        }
        for l in range(1, 5):
            m[f"Wl{l}"] = np.asarray(inputs[f"Wl{l}"], np.float32)
            m[f"Wr{l}"] = np.asarray(inputs[f"Wr{l}"], np.float32)
            m[f"b{l}"] = np.asarray(inputs[f"b{l}"], np.float32).reshape(1, -1)
        in_maps.append(m)
    return nc, in_maps, perm


def _run(inputs, trace=False):
    from concourse.bass_utils import run_bass_kernel_spmd

    nc, in_maps, perm = _prepare(inputs)
    r = run_bass_kernel_spmd(nc, in_maps, list(range(NCORES)), trace=trace)
    out = np.empty((NPAD, FOUT), np.float32)
    for k in range(NCORES):
        out[perm[k]] = np.asarray(r.results[k]["out_shard"], np.float32)
    return out[:N], r


def kernel(**inputs) -> np.ndarray:
    out, _ = _run(inputs)
    return out
